# revision 40
# baseline (speedup 1.0000x reference)
"""MixtureLinearAttention TRN2 kernel (8 NeuronCores, SPMD).

Math (per batch n, component c, head h):
  Qf = elu(q @ W_c) + 1 ;  Kf = (elu(k @ W_c) + 1) * mask
  KVt[e, m] = sum_s Kf[s,e] V[s,m] ;  Ksum[e] = sum_s Kf[s,e]
  Den[s] = sum_e Qf[s,e] Ksum[e]
  out[s,h,m] = sum_c softmax(mix)_c / Den[s] * sum_e Qf[s,e] KVt[e,m]

Sharding: core i -> (n = i//2, heads hh = (i%2)*8..+8). Host does all layout
transposes (q/k fed d-major; output returned m-major and transposed back).

v6 notes vs baseline:
- Matmul operands fp16 (halves ldweights cost; PSUM stays fp32). KV stack
  scaled by ETA=2^-12 at the psum->sbuf copy so z=1/den and zq stay in fp16
  normal range; ETA cancels exactly in the out matmul.
- PSUM-draining copies ride the scalar engine (ACT reads PSUM at full rate);
  DVE keeps the elu finish (fp16 min + scalar_tensor_tensor) and zq.
- KVt transpose matmuls write c-pair stacks at partition offset 64 directly;
  out psum packed [128,512] (head j at partition 64*j).
- Software-pipelined emission: phase C (den/z/zrep/zq/out) of h-pair hp is
  interleaved with phase A (phi-K/KV) of hp+1 so each engine's in-order
  stream always has runnable work (engines execute their streams in order).
"""
import sys

if "/opt/trn_rl_repo" not in sys.path:
    sys.path.insert(0, "/opt/trn_rl_repo")

from contextlib import ExitStack

import numpy as np

import concourse.bass as bass
import concourse.tile as tile
from concourse import mybir
from concourse.masks import make_identity

F32 = mybir.dt.float32
F16 = mybir.dt.float16
ALU = mybir.AluOpType
AFT = mybir.ActivationFunctionType

N, S, H, D, C = 4, 2048, 16, 64, 4
E = M = 64
HL = 8          # heads per core
NHP = HL // 2   # h-pairs
NCHUNK = S // 128
ETA = 2.0 ** -12


def _split_multiwait(nc, max_waits=1):
    """This walrus build rejects >1 sync wait per instruction; hoist extra
    waits onto NoOps inserted just before, on the same engine."""
    k = 0
    for fn in nc.m.functions:
        for bb in fn.blocks:
            out, changed = [], False
            for inst in bb.instructions:
                si = inst.sync_info
                if si is not None and si.on_wait and len(si.on_wait) > max_waits:
                    waits = list(si.on_wait)
                    while len(waits) > max_waits:
                        chunk, waits = waits[:max_waits], waits[max_waits:]
                        nop = mybir.InstNoOp(name=f"wait_split_{k}", ins=[], outs=[])
                        k += 1
                        nop.engine = inst.engine
                        nop.sync_info = mybir.SyncInfo(on_wait=chunk, on_update=[])
                        out.append(nop)
                        changed = True
                    inst.sync_info = mybir.SyncInfo(
                        on_wait=waits, on_update=list(si.on_update or [])
                    )
                out.append(inst)
            if changed:
                bb.instructions = out


def build_program():
    nc = bass.Bass("TRN2", debug=False)
    ap = {}
    ap["qTd"] = nc.dram_tensor("qTd", [HL, 128, S], F16, kind="ExternalInput").ap()
    ap["kT"] = nc.dram_tensor("kT", [NHP, 128, S], F16, kind="ExternalInput").ap()
    ap["vaug"] = nc.dram_tensor("vaug", [HL, 128, NCHUNK * 65], F16, kind="ExternalInput").ap()
    ap["wq2"] = nc.dram_tensor("wq2", [2, 128, 128], F16, kind="ExternalInput").ap()
    ap["wk"] = nc.dram_tensor("wk", [128, 512], F16, kind="ExternalInput").ap()
    ap["wmask"] = nc.dram_tensor("wmask", [128, 6], F32, kind="ExternalInput").ap()
    ap["sel2"] = nc.dram_tensor("sel2", [4, 128, 128], F16, kind="ExternalInput").ap()
    ap["outT"] = nc.dram_tensor("outT", [HL, 64, S], F32, kind="ExternalOutput").ap()

    tc = tile.TileContext(nc)
    with tc:
        with ExitStack() as ctx:
            cpool = ctx.enter_context(tc.tile_pool(name="consts", bufs=1))
            wq_t = []
            for p in range(2):
                w1 = cpool.tile([128, 128], F16, name=f"wq{p}", tag=f"wq{p}")
                nc.sync.dma_start(w1[:], ap["wq2"][p])
                wq_t.append(w1)
            wk_t = cpool.tile([128, 512], F16)
            nc.sync.dma_start(wk_t[:], ap["wk"][:])
            wmask_t = cpool.tile([128, 6], F32)
            nc.sync.dma_start(wmask_t[:], ap["wmask"][:])
            sel_t = []
            for i in range(4):
                s1 = cpool.tile([128, 128], F16, name=f"sel{i}", tag=f"sel{i}")
                nc.sync.dma_start(s1[:], ap["sel2"][i])
                sel_t.append(s1)
            ident = cpool.tile([128, 128], F16)
            make_identity(nc, ident[:])

            qd_pool = ctx.enter_context(tc.tile_pool(name="qd", bufs=1))
            kt_pool = ctx.enter_context(tc.tile_pool(name="kt", bufs=2))
            v_pool = ctx.enter_context(tc.tile_pool(name="v", bufs=2))
            kf_pool = ctx.enter_context(tc.tile_pool(name="kf", bufs=8))
            etK_pool = ctx.enter_context(tc.tile_pool(name="etK", bufs=2))
            etQ_pool = ctx.enter_context(tc.tile_pool(name="etQ", bufs=2))
            qf_pool = ctx.enter_context(tc.tile_pool(name="qf", bufs=5))
            kvs_pool = ctx.enter_context(tc.tile_pool(name="kvs", bufs=2))
            dl_pool = ctx.enter_context(tc.tile_pool(name="dl", bufs=2))
            z_pool = ctx.enter_context(tc.tile_pool(name="z", bufs=2))
            lnt_pool = ctx.enter_context(tc.tile_pool(name="lnt", bufs=2))
            zq_pool = ctx.enter_context(tc.tile_pool(name="zq", bufs=2))
            ob_pool = ctx.enter_context(tc.tile_pool(name="ob", bufs=3))
            # PSUM (8 banks): bigK 2 + bigQ 2 + kv 1 + sm(tp/den/out) 2 + zrep 1
            ps_bigK = ctx.enter_context(tc.tile_pool(name="psbigK", bufs=1, space="PSUM"))
            ps_bigQ = ctx.enter_context(tc.tile_pool(name="psbigQ", bufs=1, space="PSUM"))
            ps_kv = ctx.enter_context(tc.tile_pool(name="pskv", bufs=1, space="PSUM"))
            ps_sm = ctx.enter_context(tc.tile_pool(name="pssm", bufs=2, space="PSUM"))
            ps_zrep = ctx.enter_context(tc.tile_pool(name="pszrep", bufs=1, space="PSUM"))

            st = [dict() for _ in range(NHP)]

            def emit_dmas(hp):
                h0 = 2 * hp
                kt_t = kt_pool.tile([128, S], F16, name="kt")
                nc.sync.dma_start(kt_t[:], ap["kT"][hp])
                v_ts, qd_ts = [], []
                for j in range(2):
                    v_t = v_pool.tile([128, NCHUNK * 65], F16, name=f"v{j}", tag=f"v{j}")
                    nc.sync.dma_start(v_t[:], ap["vaug"][h0 + j])
                    v_ts.append(v_t)
                for j in range(2):
                    qd_t = qd_pool.tile([128, S], F16, name=f"qd{j}", tag=f"qd{j}")
                    nc.sync.dma_start(qd_t[:], ap["qTd"][h0 + j])
                    qd_ts.append(qd_t)
                st[hp].update(kt=kt_t, v=v_ts, qd=qd_ts)

            def a_blocks(hp):
                s_ = st[hp]
                s_["kv_ps"] = None
                s_["kf"] = {}

                def blk(g):
                    def go():
                        if s_["kv_ps"] is None:
                            s_["kv_ps"] = ps_kv.tile([65, 512], F32, name="kvps", tag="kvps")
                        kv_ps = s_["kv_ps"]
                        kt_t, v_ts = s_["kt"], s_["v"]
                        kf_ps = ps_bigK.tile([128, 1024], F32, name="kfps", tag="bigK")
                        for cc in range(2):
                            ch = 2 * g + cc
                            nc.tensor.matmul(
                                kf_ps[:, 512 * cc : 512 * cc + 512],
                                kt_t[:, 128 * ch : 128 * ch + 128],
                                wk_t[:],
                                start=True,
                                stop=True,
                            )
                        ek = etK_pool.tile([128, 1024], F16, name="ek", tag="ek")
                        nc.scalar.activation(ek[:], kf_ps[:], AFT.Exp)
                        akk = etK_pool.tile([128, 1024], F16, name="akk", tag="akk")
                        nc.vector.tensor_scalar(akk[:], ek[:], 1.0, None, ALU.min)
                        kf = kf_pool.tile([128, 1024], F16, name="kf")
                        if g in (2, 5):
                            # B-variant: ACT drains the psum relu; DVE add is
                            # all-SBUF (2 elem/lane/cycle) - balances ACT/DVE
                            rk = etK_pool.tile([128, 1024], F16, name="rk", tag="rk")
                            nc.scalar.activation(rk[:], kf_ps[:], AFT.Relu)
                            nc.vector.tensor_tensor(kf[:], akk[:], rk[:], ALU.add)
                        else:
                            nc.vector.scalar_tensor_tensor(
                                kf[:], kf_ps[:], 0.0, akk[:], ALU.max, ALU.add
                            )
                        for j in range(2):
                            for cc in range(2):
                                ch = 2 * g + cc
                                nc.tensor.matmul(
                                    kv_ps[:, 256 * j : 256 * j + 256],
                                    v_ts[j][:, 65 * ch : 65 * ch + 65],
                                    kf[:, 512 * cc + 256 * j : 512 * cc + 256 * j + 256],
                                    start=(ch == 0 and j == 0),
                                    stop=(ch == NCHUNK - 1),
                                )
                    return go

                return [blk(g) for g in range(8)]

            def emit_kvt_dl(hp):
                s_ = st[hp]
                kv_ps = s_["kv_ps"]
                kstk = {}
                for j in range(2):
                    kvc_t = kvs_pool.tile([65, 256], F16, name=f"kvc{j}", tag=f"kvc{j}")
                    nc.scalar.mul(kvc_t[:], kv_ps[:, 256 * j : 256 * j + 256], ETA)
                    tp_ps = ps_sm.tile([128, 130], F32, name="tpps", tag="sm")
                    for c in range(C):
                        p, odd = c // 2, c % 2
                        nc.tensor.matmul(
                            tp_ps[64 * odd : 64 * odd + 64, 65 * p : 65 * p + 65],
                            kvc_t[:, 64 * c : 64 * c + 64],
                            ident[:65, :65],
                            start=True,
                            stop=True,
                        )
                    kvs_t = kvs_pool.tile([128, 130], F16, name=f"kvs{j}", tag=f"kvs{j}")
                    nc.scalar.copy(kvs_t[:], tp_ps[:])
                    for p in range(2):
                        kstk[(j, p)] = kvs_t[:, 65 * p : 65 * p + 65]
                s_["kstk"] = kstk

                dlw = {}
                for j in range(2):
                    for p in range(2):
                        dl = dl_pool.tile([128, 128], F16, name=f"dl{j}{p}", tag=f"dl{j}{p}")
                        nc.vector.memset(dl[:], 0.0)
                        base = 4 * j + 2 * p
                        ks = kstk[(j, p)][:, 64:65]
                        nc.vector.tensor_scalar(
                            dl[:, base : base + 1], ks, wmask_t[:, 3 * p : 3 * p + 1],
                            None, ALU.mult,
                        )
                        nc.vector.tensor_scalar(
                            dl[:, base + 1 : base + 2], ks,
                            wmask_t[:, 3 * p + 1 : 3 * p + 2], None, ALU.mult,
                        )
                        if j == 0 and p == 0:
                            nc.vector.tensor_scalar(
                                dl[:, 8:128], ks.to_broadcast((128, 120)),
                                wmask_t[:, 2:3], None, ALU.mult,
                            )
                        dlw[(j, p)] = dl
                s_["dlw"] = dlw

            def b_blocks(hp):
                s_ = st[hp]
                s_["qf"] = {}

                def blk(j, p, half):
                    def go():
                        if (j, p) not in s_["qf"]:
                            s_["qf"][(j, p)] = qf_pool.tile(
                                [128, S], F16, name=f"qf{j}{p}", tag="qf"
                            )
                        qf_t = s_["qf"][(j, p)]
                        pq_ps = ps_bigQ.tile([128, 1024], F32, name="pqps", tag="bigQ")
                        for g in range(2):
                            o = 1024 * half + 512 * g
                            nc.tensor.matmul(
                                pq_ps[:, 512 * g : 512 * g + 512],
                                wq_t[p][:],
                                s_["qd"][j][:, o : o + 512],
                                start=True,
                                stop=True,
                            )
                        eq = etQ_pool.tile([128, 1024], F16, name="eq", tag="eq")
                        nc.scalar.activation(eq[:], pq_ps[:], AFT.Exp)
                        aq = etQ_pool.tile([128, 1024], F16, name="aq", tag="aq")
                        nc.vector.tensor_scalar(aq[:], eq[:], 1.0, None, ALU.min)
                        if 4 * j + 2 * p + half in (2, 5):
                            rq = etQ_pool.tile([128, 1024], F16, name="rq", tag="rq")
                            nc.scalar.activation(rq[:], pq_ps[:], AFT.Relu)
                            nc.vector.tensor_tensor(
                                qf_t[:, 1024 * half : 1024 * half + 1024],
                                aq[:], rq[:], ALU.add,
                            )
                        else:
                            nc.vector.scalar_tensor_tensor(
                                qf_t[:, 1024 * half : 1024 * half + 1024],
                                pq_ps[:], 0.0, aq[:], ALU.max, ALU.add,
                            )
                    return go

                return [blk(j, p, half) for j in range(2) for p in range(2)
                        for half in range(2)]

            def c_blocks(hp):
                s_ = st[hp]
                h0 = 2 * hp

                def blk(ch4):
                    def go():
                        sl = slice(512 * ch4, 512 * ch4 + 512)
                        qf2, dlw, kstk = s_["qf"], s_["dlw"], s_["kstk"]
                        den_ps = ps_sm.tile([128, 512], F32, name="denps", tag="sm")
                        for i, (j, p) in enumerate(((0, 0), (0, 1), (1, 0), (1, 1))):
                            nc.tensor.matmul(
                                den_ps[:],
                                dlw[(j, p)][:],
                                qf2[(j, p)][:, sl],
                                start=(i == 0),
                                stop=(i == 3),
                            )
                        lnt = lnt_pool.tile([128, 512], F32, name="lnt", tag="lnt")
                        nc.scalar.activation(lnt[:], den_ps[:], AFT.Ln)
                        z_t = z_pool.tile([128, 512], F16, name="zt", tag="z")
                        nc.scalar.activation(z_t[:], lnt[:], AFT.Exp, scale=-1.0)

                        ot_ps = ps_sm.tile([128, 512], F32, name="otps", tag="sm")
                        for j in range(2):
                            for p in range(2):
                                zrep_ps = ps_zrep.tile([128, 512], F32, name="zrepps", tag="zrep")
                                nc.tensor.matmul(
                                    zrep_ps[:], sel_t[2 * j + p][:], z_t[:],
                                    start=True, stop=True,
                                )
                                zq_t = zq_pool.tile([128, 512], F16, name="zqt", tag="zq")
                                nc.vector.tensor_tensor(
                                    zq_t[:], qf2[(j, p)][:, sl], zrep_ps[:], ALU.mult
                                )
                                nc.tensor.matmul(
                                    ot_ps[64 * j : 64 * j + 64, :],
                                    kstk[(j, p)][:, 0:64],
                                    zq_t[:],
                                    start=(p == 0),
                                    stop=(p == 1),
                                )
                        ob5 = ob_pool.tile([128, 512], F32, name="ob5", tag="ob")
                        nc.scalar.copy(ob5[:], ot_ps[:])
                        for j in range(2):
                            nc.sync.dma_start(
                                ap["outT"][h0 + j][:, sl], ob5[64 * j : 64 * j + 64, :]
                            )
                    return go

                return [blk(ch4) for ch4 in range(4)]

            # ---- pipelined emission: interleave C(hp) with A(hp+1) so each
            # engine's in-order stream always has runnable work ----
            emit_dmas(0)
            for b in a_blocks(0):
                b()
            emit_kvt_dl(0)
            for b in b_blocks(0):
                b()
            for hp in range(NHP):
                nxt = hp + 1
                if nxt < NHP:
                    emit_dmas(nxt)
                    ab = a_blocks(nxt)
                else:
                    ab = []
                ai = iter(ab)
                for c in c_blocks(hp):
                    for _ in range(2):
                        a = next(ai, None)
                        if a:
                            a()
                    c()
                for a in ai:
                    a()
                if nxt < NHP:
                    emit_kvt_dl(nxt)
                    for b in b_blocks(nxt):
                        b()

    _split_multiwait(nc)
    return nc


_NC_CACHE = None


def _get_nc():
    global _NC_CACHE
    if _NC_CACHE is None:
        _NC_CACHE = build_program()
    return _NC_CACHE


def _softmax(x):
    e = np.exp(x - x.max())
    return e / e.sum()


def prep_core_inputs(queries, keys, values, key_mask, feat_W, mix_weights, core):
    n, hh = core // 2, (core % 2) * HL
    W = _softmax(np.asarray(mix_weights, np.float64)).astype(np.float32)

    qs = queries[n][:, hh : hh + HL, :].transpose(1, 2, 0)  # [HL, D, S]
    qTd = np.ascontiguousarray(
        np.concatenate([qs, qs], axis=1)
    ).astype(np.float16)  # [HL, 128, S]
    ks = keys[n][:, hh : hh + HL, :].transpose(1, 2, 0)
    kT = np.ascontiguousarray(ks.reshape(NHP, 128, S)).astype(np.float16)

    mask = key_mask[n].astype(np.float32)
    vm = values[n][:, hh : hh + HL, :] * mask[:, None, None]
    vaug = np.concatenate(
        [vm, np.broadcast_to(mask[:, None, None], (S, HL, 1))], axis=2
    )
    vaug = vaug.transpose(1, 0, 2).reshape(HL, NCHUNK, 128, 65)
    vaug = np.ascontiguousarray(vaug.transpose(0, 2, 1, 3)).reshape(
        HL, 128, NCHUNK * 65
    ).astype(np.float16)

    wq2 = np.zeros((2, 128, 128), np.float16)
    for p in range(2):
        wq2[p, :64, :64] = feat_W[2 * p]
        wq2[p, 64:, 64:] = feat_W[2 * p + 1]
    wcat = np.concatenate([feat_W[c] for c in range(C)], axis=1)
    wk = np.zeros((128, 512), np.float16)
    wk[:64, :256] = wcat
    wk[64:, 256:] = wcat

    wmask = np.zeros((128, 6), np.float32)
    for p in range(2):
        wmask[:64, 3 * p + 0] = 1.0 / W[2 * p]
        wmask[64:, 3 * p + 1] = 1.0 / W[2 * p + 1]
        wmask[:64, 3 * p + 2] = 1.0 / W[2 * p]
        wmask[64:, 3 * p + 2] = 1.0 / W[2 * p + 1]

    sel2 = np.zeros((4, 128, 128), np.float16)
    for j in range(2):
        for p in range(2):
            base = 4 * j + 2 * p
            sel2[2 * j + p, base, :64] = 1.0
            sel2[2 * j + p, base + 1, 64:] = 1.0

    return {"qTd": qTd, "kT": kT, "vaug": vaug, "wq2": wq2, "wk": wk,
            "wmask": wmask, "sel2": sel2}


def run_cores(inputs, trace=False, tmpdir=None):
    from concourse.bass_utils import run_bass_kernel_spmd

    nc = _get_nc()
    in_maps = [prep_core_inputs(**inputs, core=i) for i in range(8)]
    kwargs = {}
    if trace:
        kwargs = {"trace": True, "tmpdir": tmpdir}
    res = run_bass_kernel_spmd(nc, in_maps, core_ids=list(range(8)), **kwargs)
    out = np.empty((N, S, H, M), np.float32)
    for i in range(8):
        n, hh = i // 2, (i % 2) * HL
        oT = res.results[i]["outT"]  # [HL, 64, S]
        for h in range(HL):
            out[n, :, hh + h, :] = oT[h].T
    return out, res


def kernel(queries, keys, values, key_mask, feat_W, mix_weights):
    out, _ = run_cores(
        dict(queries=np.asarray(queries), keys=np.asarray(keys),
             values=np.asarray(values), key_mask=np.asarray(key_mask),
             feat_W=np.asarray(feat_W), mix_weights=np.asarray(mix_weights))
    )
    return out


# revision 43
# speedup vs baseline: 1.0405x; 1.0405x over previous
"""MixtureLinearAttention TRN2 kernel (8 NeuronCores, SPMD).

Math (per batch n, component c, head h):
  Qf = elu(q @ W_c) + 1 ;  Kf = (elu(k @ W_c) + 1) * mask
  KVt[e, m] = sum_s Kf[s,e] V[s,m] ;  Ksum[e] = sum_s Kf[s,e]
  Den[s] = sum_e Qf[s,e] Ksum[e]
  out[s,h,m] = sum_c softmax(mix)_c / Den[s] * sum_e Qf[s,e] KVt[e,m]

Sharding: core i -> (n = i//2, heads hh = (i%2)*8..+8). Host does all layout
transposes (q/k fed d-major; output returned m-major and transposed back).

v6 notes vs baseline:
- Matmul operands fp16 (halves ldweights cost; PSUM stays fp32). KV stack
  scaled by ETA=2^-12 at the psum->sbuf copy so z=1/den and zq stay in fp16
  normal range; ETA cancels exactly in the out matmul.
- PSUM-draining copies ride the scalar engine (ACT reads PSUM at full rate);
  DVE keeps the elu finish (fp16 min + scalar_tensor_tensor) and zq.
- KVt transpose matmuls write c-pair stacks at partition offset 64 directly;
  out psum packed [128,512] (head j at partition 64*j).
- Software-pipelined emission: phase C (den/z/zrep/zq/out) of h-pair hp is
  interleaved with phase A (phi-K/KV) of hp+1 so each engine's in-order
  stream always has runnable work (engines execute their streams in order).
"""
import sys

if "/opt/trn_rl_repo" not in sys.path:
    sys.path.insert(0, "/opt/trn_rl_repo")

from contextlib import ExitStack

import numpy as np

import concourse.bass as bass
import concourse.tile as tile
from concourse import mybir
from concourse.masks import make_identity

F32 = mybir.dt.float32
F16 = mybir.dt.float16
ALU = mybir.AluOpType
AFT = mybir.ActivationFunctionType

N, S, H, D, C = 4, 2048, 16, 64, 4
E = M = 64
HL = 8          # heads per core
NHP = HL // 2   # h-pairs
NCHUNK = S // 128
ETA = 2.0 ** -12


def _split_multiwait(nc, max_waits=1):
    """This walrus build rejects >1 sync wait per instruction; hoist extra
    waits onto NoOps inserted just before, on the same engine."""
    k = 0
    for fn in nc.m.functions:
        for bb in fn.blocks:
            out, changed = [], False
            for inst in bb.instructions:
                si = inst.sync_info
                if si is not None and si.on_wait and len(si.on_wait) > max_waits:
                    waits = list(si.on_wait)
                    while len(waits) > max_waits:
                        chunk, waits = waits[:max_waits], waits[max_waits:]
                        nop = mybir.InstNoOp(name=f"wait_split_{k}", ins=[], outs=[])
                        k += 1
                        nop.engine = inst.engine
                        nop.sync_info = mybir.SyncInfo(on_wait=chunk, on_update=[])
                        out.append(nop)
                        changed = True
                    inst.sync_info = mybir.SyncInfo(
                        on_wait=waits, on_update=list(si.on_update or [])
                    )
                out.append(inst)
            if changed:
                bb.instructions = out


def build_program():
    nc = bass.Bass("TRN2", debug=False)
    ap = {}
    ap["qTd"] = nc.dram_tensor("qTd", [HL, 128, S], F16, kind="ExternalInput").ap()
    ap["kT"] = nc.dram_tensor("kT", [NHP, 128, S], F16, kind="ExternalInput").ap()
    ap["vaug"] = nc.dram_tensor("vaug", [HL, 128, NCHUNK * 65], F16, kind="ExternalInput").ap()
    ap["wq2"] = nc.dram_tensor("wq2", [2, 128, 128], F16, kind="ExternalInput").ap()
    ap["wk"] = nc.dram_tensor("wk", [128, 512], F16, kind="ExternalInput").ap()
    ap["wmask"] = nc.dram_tensor("wmask", [128, 6], F32, kind="ExternalInput").ap()
    ap["sel2"] = nc.dram_tensor("sel2", [4, 128, 128], F16, kind="ExternalInput").ap()
    ap["outT"] = nc.dram_tensor("outT", [HL, 64, S], F32, kind="ExternalOutput").ap()

    tc = tile.TileContext(nc)
    with tc:
        with ExitStack() as ctx:
            cpool = ctx.enter_context(tc.tile_pool(name="consts", bufs=1))
            wq_t = []
            for p in range(2):
                w1 = cpool.tile([128, 128], F16, name=f"wq{p}", tag=f"wq{p}")
                nc.sync.dma_start(w1[:], ap["wq2"][p])
                wq_t.append(w1)
            wk_t = cpool.tile([128, 512], F16)
            nc.sync.dma_start(wk_t[:], ap["wk"][:])

            qd_pool = ctx.enter_context(tc.tile_pool(name="qd", bufs=1))
            kt_pool = ctx.enter_context(tc.tile_pool(name="kt", bufs=2))
            v_pool = ctx.enter_context(tc.tile_pool(name="v", bufs=2))
            kf_pool = ctx.enter_context(tc.tile_pool(name="kf", bufs=8))
            etK_pool = ctx.enter_context(tc.tile_pool(name="etK", bufs=2))
            etQ_pool = ctx.enter_context(tc.tile_pool(name="etQ", bufs=2))
            qf_pool = ctx.enter_context(tc.tile_pool(name="qf", bufs=5))
            kvs_pool = ctx.enter_context(tc.tile_pool(name="kvs", bufs=2))
            dl_pool = ctx.enter_context(tc.tile_pool(name="dl", bufs=2))
            z_pool = ctx.enter_context(tc.tile_pool(name="z", bufs=2))
            lnt_pool = ctx.enter_context(tc.tile_pool(name="lnt", bufs=2))
            zq_pool = ctx.enter_context(tc.tile_pool(name="zq", bufs=2))
            ob_pool = ctx.enter_context(tc.tile_pool(name="ob", bufs=3))
            # PSUM (8 banks): bigK 2 + bigQ 2 + kv 1 + sm 2 + out 1
            ps_bigK = ctx.enter_context(tc.tile_pool(name="psbigK", bufs=1, space="PSUM"))
            ps_bigQ = ctx.enter_context(tc.tile_pool(name="psbigQ", bufs=1, space="PSUM"))
            ps_kv = ctx.enter_context(tc.tile_pool(name="pskv", bufs=1, space="PSUM"))
            ps_sm = ctx.enter_context(tc.tile_pool(name="pssm", bufs=2, space="PSUM"))
            ps_out = ctx.enter_context(tc.tile_pool(name="psout", bufs=1, space="PSUM"))

            st = [dict() for _ in range(NHP)]

            def emit_dmas(hp):
                h0 = 2 * hp
                kt_t = kt_pool.tile([128, S], F16, name="kt")
                nc.sync.dma_start(kt_t[:], ap["kT"][hp])
                v_ts, qd_ts = [], []
                for j in range(2):
                    v_t = v_pool.tile([128, NCHUNK * 65], F16, name=f"v{j}", tag=f"v{j}")
                    nc.sync.dma_start(v_t[:], ap["vaug"][h0 + j])
                    v_ts.append(v_t)
                for j in range(2):
                    qd_t = qd_pool.tile([128, S], F16, name=f"qd{j}", tag=f"qd{j}")
                    nc.sync.dma_start(qd_t[:], ap["qTd"][h0 + j])
                    qd_ts.append(qd_t)
                st[hp].update(kt=kt_t, v=v_ts, qd=qd_ts)

            def a_blocks(hp):
                s_ = st[hp]
                s_["kv_ps"] = None
                s_["kf"] = {}

                def blk(g):
                    def go():
                        if s_["kv_ps"] is None:
                            s_["kv_ps"] = ps_kv.tile([65, 512], F32, name="kvps", tag="kvps")
                        kv_ps = s_["kv_ps"]
                        kt_t, v_ts = s_["kt"], s_["v"]
                        kf_ps = ps_bigK.tile([128, 1024], F32, name="kfps", tag="bigK")
                        for cc in range(2):
                            ch = 2 * g + cc
                            nc.tensor.matmul(
                                kf_ps[:, 512 * cc : 512 * cc + 512],
                                kt_t[:, 128 * ch : 128 * ch + 128],
                                wk_t[:],
                                start=True,
                                stop=True,
                            )
                        ek = etK_pool.tile([128, 1024], F16, name="ek", tag="ek")
                        nc.scalar.activation(ek[:], kf_ps[:], AFT.Exp)
                        akk = etK_pool.tile([128, 1024], F16, name="akk", tag="akk")
                        nc.vector.tensor_scalar(akk[:], ek[:], 1.0, None, ALU.min)
                        kf = kf_pool.tile([128, 1024], F16, name="kf")
                        if g in (2, 5):
                            # B-variant: ACT drains the psum relu; DVE add is
                            # all-SBUF (2 elem/lane/cycle) - balances ACT/DVE
                            rk = etK_pool.tile([128, 1024], F16, name="rk", tag="rk")
                            nc.scalar.activation(rk[:], kf_ps[:], AFT.Relu)
                            nc.vector.tensor_tensor(kf[:], akk[:], rk[:], ALU.add)
                        else:
                            nc.vector.scalar_tensor_tensor(
                                kf[:], kf_ps[:], 0.0, akk[:], ALU.max, ALU.add
                            )
                        for j in range(2):
                            for cc in range(2):
                                ch = 2 * g + cc
                                nc.tensor.matmul(
                                    kv_ps[:, 256 * j : 256 * j + 256],
                                    v_ts[j][:, 65 * ch : 65 * ch + 65],
                                    kf[:, 512 * cc + 256 * j : 512 * cc + 256 * j + 256],
                                    start=(ch == 0 and j == 0),
                                    stop=(ch == NCHUNK - 1),
                                )
                    return go

                return [blk(g) for g in range(8)]

            def emit_kvt_dl(hp):
                s_ = st[hp]
                kv_ps = s_["kv_ps"]
                kstk = {}
                for j in range(2):
                    kvc_t = kvs_pool.tile([65, 256], F16, name=f"kvc{j}", tag=f"kvc{j}")
                    nc.scalar.mul(kvc_t[:], kv_ps[:, 256 * j : 256 * j + 256], ETA)
                    tp_ps = ps_sm.tile([128, 130], F32, name="tpps", tag="sm")
                    for c in range(C):
                        p, odd = c // 2, c % 2
                        nc.tensor.matmul(
                            tp_ps[64 * odd : 64 * odd + 64, 65 * p : 65 * p + 65],
                            kvc_t[:, 64 * c : 64 * c + 64],
                            ident[:65, :65],
                            start=True,
                            stop=True,
                        )
                    kvs_t = kvs_pool.tile([128, 130], F16, name=f"kvs{j}", tag=f"kvs{j}")
                    nc.scalar.copy(kvs_t[:], tp_ps[:])
                    for p in range(2):
                        kstk[(j, p)] = kvs_t[:, 65 * p : 65 * p + 65]
                s_["kstk"] = kstk

                dlw = {}
                for j in range(2):
                    for p in range(2):
                        dl = dl_pool.tile([128, 128], F16, name=f"dl{j}{p}", tag=f"dl{j}{p}")
                        nc.vector.memset(dl[:], 0.0)
                        base = 4 * j + 2 * p
                        ks = kstk[(j, p)][:, 64:65]
                        nc.vector.tensor_scalar(
                            dl[:, base : base + 1], ks, wmask_t[:, 3 * p : 3 * p + 1],
                            None, ALU.mult,
                        )
                        nc.vector.tensor_scalar(
                            dl[:, base + 1 : base + 2], ks,
                            wmask_t[:, 3 * p + 1 : 3 * p + 2], None, ALU.mult,
                        )
                        if j == 0 and p == 0:
                            nc.vector.tensor_scalar(
                                dl[:, 8:128], ks.to_broadcast((128, 120)),
                                wmask_t[:, 2:3], None, ALU.mult,
                            )
                        dlw[(j, p)] = dl
                s_["dlw"] = dlw

            def b_blocks(hp):
                s_ = st[hp]
                s_["qf"] = {}

                def blk(j, p, half):
                    def go():
                        if (j, p) not in s_["qf"]:
                            s_["qf"][(j, p)] = qf_pool.tile(
                                [128, S], F16, name=f"qf{j}{p}", tag="qf"
                            )
                        qf_t = s_["qf"][(j, p)]
                        pq_ps = ps_bigQ.tile([128, 1024], F32, name="pqps", tag="bigQ")
                        for g in range(2):
                            o = 1024 * half + 512 * g
                            nc.tensor.matmul(
                                pq_ps[:, 512 * g : 512 * g + 512],
                                wq_t[p][:],
                                s_["qd"][j][:, o : o + 512],
                                start=True,
                                stop=True,
                            )
                        eq = etQ_pool.tile([128, 1024], F16, name="eq", tag="eq")
                        nc.scalar.activation(eq[:], pq_ps[:], AFT.Exp)
                        aq = etQ_pool.tile([128, 1024], F16, name="aq", tag="aq")
                        nc.vector.tensor_scalar(aq[:], eq[:], 1.0, None, ALU.min)
                        if 4 * j + 2 * p + half in (2, 5):
                            rq = etQ_pool.tile([128, 1024], F16, name="rq", tag="rq")
                            nc.scalar.activation(rq[:], pq_ps[:], AFT.Relu)
                            nc.vector.tensor_tensor(
                                qf_t[:, 1024 * half : 1024 * half + 1024],
                                aq[:], rq[:], ALU.add,
                            )
                        else:
                            nc.vector.scalar_tensor_tensor(
                                qf_t[:, 1024 * half : 1024 * half + 1024],
                                pq_ps[:], 0.0, aq[:], ALU.max, ALU.add,
                            )
                    return go

                return [blk(j, p, half) for j in range(2) for p in range(2)
                        for half in range(2)]

            def c_blocks(hp):
                s_ = st[hp]
                h0 = 2 * hp

                def blk(ch4):
                    def go():
                        sl = slice(512 * ch4, 512 * ch4 + 512)
                        qf2, dlw, kstk = s_["qf"], s_["dlw"], s_["kstk"]
                        den_ps = ps_sm.tile([128, 512], F32, name="denps", tag="sm")
                        for i, (j, p) in enumerate(((0, 0), (0, 1), (1, 0), (1, 1))):
                            nc.tensor.matmul(
                                den_ps[:],
                                dlw[(j, p)][:],
                                qf2[(j, p)][:, sl],
                                start=(i == 0),
                                stop=(i == 3),
                            )
                        lnt = lnt_pool.tile([128, 512], F32, name="lnt", tag="lnt")
                        nc.scalar.activation(lnt[:], den_ps[:], AFT.Ln)
                        z_t = z_pool.tile([128, 512], F16, name="zt", tag="z")
                        nc.scalar.activation(z_t[:], lnt[:], AFT.Exp, scale=-1.0)

                        ot_ps = ps_out.tile([128, 512], F32, name="otps", tag="ot")
                        for j in range(2):
                            for p in range(2):
                                zrep_ps = ps_sm.tile([128, 512], F32, name="zrepps", tag="sm")
                                nc.tensor.matmul(
                                    zrep_ps[:], sel_t[2 * j + p][:], z_t[:],
                                    start=True, stop=True,
                                )
                                zq_t = zq_pool.tile([128, 512], F16, name="zqt", tag="zq")
                                nc.vector.tensor_tensor(
                                    zq_t[:], qf2[(j, p)][:, sl], zrep_ps[:], ALU.mult
                                )
                                nc.tensor.matmul(
                                    ot_ps[64 * j : 64 * j + 64, :],
                                    kstk[(j, p)][:, 0:64],
                                    zq_t[:],
                                    start=(p == 0),
                                    stop=(p == 1),
                                )
                        ob5 = ob_pool.tile([128, 512], F32, name="ob5", tag="ob")
                        nc.scalar.copy(ob5[:], ot_ps[:])
                        for j in range(2):
                            nc.sync.dma_start(
                                ap["outT"][h0 + j][:, sl], ob5[64 * j : 64 * j + 64, :]
                            )
                    return go

                return [blk(ch4) for ch4 in range(4)]

            # ---- pipelined emission: interleave C(hp) with A(hp+1) so each
            # engine's in-order stream always has runnable work ----
            emit_dmas(0)
            # small constants after the big hp0 input DMAs so kT/vaug land
            # sooner (SP issues DMAs in order); same cpool allocation order
            wmask_t = cpool.tile([128, 6], F32)
            nc.sync.dma_start(wmask_t[:], ap["wmask"][:])
            sel_t = []
            for i in range(4):
                s1 = cpool.tile([128, 128], F16, name=f"sel{i}", tag=f"sel{i}")
                nc.sync.dma_start(s1[:], ap["sel2"][i])
                sel_t.append(s1)
            ident = cpool.tile([128, 128], F16)
            make_identity(nc, ident[:])
            for b in a_blocks(0):
                b()
            emit_kvt_dl(0)
            for b in b_blocks(0):
                b()
            for hp in range(NHP):
                nxt = hp + 1
                if nxt < NHP:
                    emit_dmas(nxt)
                    ab = a_blocks(nxt)
                else:
                    ab = []
                ai = iter(ab)
                for c in c_blocks(hp):
                    for _ in range(2):
                        a = next(ai, None)
                        if a:
                            a()
                    c()
                for a in ai:
                    a()
                if nxt < NHP:
                    emit_kvt_dl(nxt)
                    for b in b_blocks(nxt):
                        b()

    _split_multiwait(nc)
    return nc


_NC_CACHE = None


def _get_nc():
    global _NC_CACHE
    if _NC_CACHE is None:
        _NC_CACHE = build_program()
    return _NC_CACHE


def _softmax(x):
    e = np.exp(x - x.max())
    return e / e.sum()


def prep_core_inputs(queries, keys, values, key_mask, feat_W, mix_weights, core):
    n, hh = core // 2, (core % 2) * HL
    W = _softmax(np.asarray(mix_weights, np.float64)).astype(np.float32)

    qs = queries[n][:, hh : hh + HL, :].transpose(1, 2, 0)  # [HL, D, S]
    qTd = np.ascontiguousarray(
        np.concatenate([qs, qs], axis=1)
    ).astype(np.float16)  # [HL, 128, S]
    ks = keys[n][:, hh : hh + HL, :].transpose(1, 2, 0)
    kT = np.ascontiguousarray(ks.reshape(NHP, 128, S)).astype(np.float16)

    mask = key_mask[n].astype(np.float32)
    vm = values[n][:, hh : hh + HL, :] * mask[:, None, None]
    vaug = np.concatenate(
        [vm, np.broadcast_to(mask[:, None, None], (S, HL, 1))], axis=2
    )
    vaug = vaug.transpose(1, 0, 2).reshape(HL, NCHUNK, 128, 65)
    vaug = np.ascontiguousarray(vaug.transpose(0, 2, 1, 3)).reshape(
        HL, 128, NCHUNK * 65
    ).astype(np.float16)

    wq2 = np.zeros((2, 128, 128), np.float16)
    for p in range(2):
        wq2[p, :64, :64] = feat_W[2 * p]
        wq2[p, 64:, 64:] = feat_W[2 * p + 1]
    wcat = np.concatenate([feat_W[c] for c in range(C)], axis=1)
    wk = np.zeros((128, 512), np.float16)
    wk[:64, :256] = wcat
    wk[64:, 256:] = wcat

    wmask = np.zeros((128, 6), np.float32)
    for p in range(2):
        wmask[:64, 3 * p + 0] = 1.0 / W[2 * p]
        wmask[64:, 3 * p + 1] = 1.0 / W[2 * p + 1]
        wmask[:64, 3 * p + 2] = 1.0 / W[2 * p]
        wmask[64:, 3 * p + 2] = 1.0 / W[2 * p + 1]

    sel2 = np.zeros((4, 128, 128), np.float16)
    for j in range(2):
        for p in range(2):
            base = 4 * j + 2 * p
            sel2[2 * j + p, base, :64] = 1.0
            sel2[2 * j + p, base + 1, 64:] = 1.0

    return {"qTd": qTd, "kT": kT, "vaug": vaug, "wq2": wq2, "wk": wk,
            "wmask": wmask, "sel2": sel2}


def run_cores(inputs, trace=False, tmpdir=None):
    from concourse.bass_utils import run_bass_kernel_spmd

    nc = _get_nc()
    in_maps = [prep_core_inputs(**inputs, core=i) for i in range(8)]
    kwargs = {}
    if trace:
        kwargs = {"trace": True, "tmpdir": tmpdir}
    res = run_bass_kernel_spmd(nc, in_maps, core_ids=list(range(8)), **kwargs)
    out = np.empty((N, S, H, M), np.float32)
    for i in range(8):
        n, hh = i // 2, (i % 2) * HL
        oT = res.results[i]["outT"]  # [HL, 64, S]
        for h in range(HL):
            out[n, :, hh + h, :] = oT[h].T
    return out, res


def kernel(queries, keys, values, key_mask, feat_W, mix_weights):
    out, _ = run_cores(
        dict(queries=np.asarray(queries), keys=np.asarray(keys),
             values=np.asarray(values), key_mask=np.asarray(key_mask),
             feat_W=np.asarray(feat_W), mix_weights=np.asarray(mix_weights))
    )
    return out


# revision 45
# speedup vs baseline: 1.0700x; 1.0284x over previous
"""MixtureLinearAttention TRN2 kernel (8 NeuronCores, SPMD).

Math (per batch n, component c, head h):
  Qf = elu(q @ W_c) + 1 ;  Kf = (elu(k @ W_c) + 1) * mask
  KVt[e, m] = sum_s Kf[s,e] V[s,m] ;  Ksum[e] = sum_s Kf[s,e]
  Den[s] = sum_e Qf[s,e] Ksum[e]
  out[s,h,m] = sum_c softmax(mix)_c / Den[s] * sum_e Qf[s,e] KVt[e,m]

Sharding: core i -> (n = i//2, heads hh = (i%2)*8..+8). Host does all layout
transposes (q/k fed d-major; output returned m-major and transposed back).

v6 notes vs baseline:
- Matmul operands fp16 (halves ldweights cost; PSUM stays fp32). KV stack
  scaled by ETA=2^-12 at the psum->sbuf copy so z=1/den and zq stay in fp16
  normal range; ETA cancels exactly in the out matmul.
- PSUM-draining copies ride the scalar engine (ACT reads PSUM at full rate);
  DVE keeps the elu finish (fp16 min + scalar_tensor_tensor) and zq.
- KVt transpose matmuls write c-pair stacks at partition offset 64 directly;
  out psum packed [128,512] (head j at partition 64*j).
- Software-pipelined emission: phase C (den/z/zrep/zq/out) of h-pair hp is
  interleaved with phase A (phi-K/KV) of hp+1 so each engine's in-order
  stream always has runnable work (engines execute their streams in order).
"""
import sys

if "/opt/trn_rl_repo" not in sys.path:
    sys.path.insert(0, "/opt/trn_rl_repo")

from contextlib import ExitStack

import numpy as np

import concourse.bass as bass
import concourse.tile as tile
from concourse import mybir
from concourse.masks import make_identity

F32 = mybir.dt.float32
F16 = mybir.dt.float16
ALU = mybir.AluOpType
AFT = mybir.ActivationFunctionType

N, S, H, D, C = 4, 2048, 16, 64, 4
E = M = 64
HL = 8          # heads per core
NHP = HL // 2   # h-pairs
NCHUNK = S // 128
ETA = 2.0 ** -12


def _split_multiwait(nc, max_waits=1):
    """This walrus build rejects >1 sync wait per instruction; hoist extra
    waits onto NoOps inserted just before, on the same engine."""
    k = 0
    for fn in nc.m.functions:
        for bb in fn.blocks:
            out, changed = [], False
            for inst in bb.instructions:
                si = inst.sync_info
                if si is not None and si.on_wait and len(si.on_wait) > max_waits:
                    waits = list(si.on_wait)
                    while len(waits) > max_waits:
                        chunk, waits = waits[:max_waits], waits[max_waits:]
                        nop = mybir.InstNoOp(name=f"wait_split_{k}", ins=[], outs=[])
                        k += 1
                        nop.engine = inst.engine
                        nop.sync_info = mybir.SyncInfo(on_wait=chunk, on_update=[])
                        out.append(nop)
                        changed = True
                    inst.sync_info = mybir.SyncInfo(
                        on_wait=waits, on_update=list(si.on_update or [])
                    )
                out.append(inst)
            if changed:
                bb.instructions = out


def build_program():
    nc = bass.Bass("TRN2", debug=False)
    ap = {}
    ap["qTd"] = nc.dram_tensor("qTd", [HL, 128, S], F16, kind="ExternalInput").ap()
    ap["kT"] = nc.dram_tensor("kT", [NHP, 128, S], F16, kind="ExternalInput").ap()
    ap["vaug"] = nc.dram_tensor("vaug", [HL, 128, NCHUNK * 65], F16, kind="ExternalInput").ap()
    ap["wq2"] = nc.dram_tensor("wq2", [2, 128, 128], F16, kind="ExternalInput").ap()
    ap["wk"] = nc.dram_tensor("wk", [128, 512], F16, kind="ExternalInput").ap()
    ap["wmask"] = nc.dram_tensor("wmask", [128, 6], F32, kind="ExternalInput").ap()
    ap["sel2"] = nc.dram_tensor("sel2", [4, 128, 128], F16, kind="ExternalInput").ap()
    ap["outT"] = nc.dram_tensor("outT", [HL, 64, S], F32, kind="ExternalOutput").ap()

    tc = tile.TileContext(nc)
    with tc:
        with ExitStack() as ctx:
            cpool = ctx.enter_context(tc.tile_pool(name="consts", bufs=1))
            wq_t = []
            for p in range(2):
                w1 = cpool.tile([128, 128], F16, name=f"wq{p}", tag=f"wq{p}")
                nc.sync.dma_start(w1[:], ap["wq2"][p])
                wq_t.append(w1)
            wk_t = cpool.tile([128, 512], F16)
            nc.sync.dma_start(wk_t[:], ap["wk"][:])
            wmask_t = cpool.tile([128, 6], F32)
            nc.sync.dma_start(wmask_t[:], ap["wmask"][:])
            sel_t = []
            for i in range(4):
                s1 = cpool.tile([128, 128], F16, name=f"sel{i}", tag=f"sel{i}")
                nc.sync.dma_start(s1[:], ap["sel2"][i])
                sel_t.append(s1)
            ident = cpool.tile([128, 128], F16)
            make_identity(nc, ident[:])

            qd_pool = ctx.enter_context(tc.tile_pool(name="qd", bufs=1))
            kt_pool = ctx.enter_context(tc.tile_pool(name="kt", bufs=2))
            v_pool = ctx.enter_context(tc.tile_pool(name="v", bufs=2))
            kf_pool = ctx.enter_context(tc.tile_pool(name="kf", bufs=8))
            etK_pool = ctx.enter_context(tc.tile_pool(name="etK", bufs=2))
            etQ_pool = ctx.enter_context(tc.tile_pool(name="etQ", bufs=2))
            qf_pool = ctx.enter_context(tc.tile_pool(name="qf", bufs=5))
            kvs_pool = ctx.enter_context(tc.tile_pool(name="kvs", bufs=2))
            dl_pool = ctx.enter_context(tc.tile_pool(name="dl", bufs=2))
            z_pool = ctx.enter_context(tc.tile_pool(name="z", bufs=2))
            lnt_pool = ctx.enter_context(tc.tile_pool(name="lnt", bufs=2))
            zq_pool = ctx.enter_context(tc.tile_pool(name="zq", bufs=2))
            ob_pool = ctx.enter_context(tc.tile_pool(name="ob", bufs=3))
            # PSUM (8 banks): bigK 2 + bigQ 2 + kv 1 + sm 2 + out 1
            ps_bigK = ctx.enter_context(tc.tile_pool(name="psbigK", bufs=1, space="PSUM"))
            ps_bigQ = ctx.enter_context(tc.tile_pool(name="psbigQ", bufs=1, space="PSUM"))
            ps_kv = ctx.enter_context(tc.tile_pool(name="pskv", bufs=1, space="PSUM"))
            ps_sm = ctx.enter_context(tc.tile_pool(name="pssm", bufs=2, space="PSUM"))
            ps_out = ctx.enter_context(tc.tile_pool(name="psout", bufs=1, space="PSUM"))

            st = [dict() for _ in range(NHP)]

            def emit_dmas(hp):
                h0 = 2 * hp
                kt_t = kt_pool.tile([128, S], F16, name="kt")
                nc.sync.dma_start(kt_t[:], ap["kT"][hp])
                v_ts, qd_ts = [], []
                for j in range(2):
                    v_t = v_pool.tile([128, NCHUNK * 65], F16, name=f"v{j}", tag=f"v{j}")
                    nc.sync.dma_start(v_t[:], ap["vaug"][h0 + j])
                    v_ts.append(v_t)
                for j in range(2):
                    qd_t = qd_pool.tile([128, S], F16, name=f"qd{j}", tag=f"qd{j}")
                    nc.sync.dma_start(qd_t[:], ap["qTd"][h0 + j])
                    qd_ts.append(qd_t)
                st[hp].update(kt=kt_t, v=v_ts, qd=qd_ts)

            def a_blocks(hp):
                s_ = st[hp]
                s_["kv_ps"] = None
                s_["kf"] = {}

                def blk(g):
                    def go():
                        if s_["kv_ps"] is None:
                            s_["kv_ps"] = ps_kv.tile([65, 512], F32, name="kvps", tag="kvps")
                        kv_ps = s_["kv_ps"]
                        kt_t, v_ts = s_["kt"], s_["v"]
                        kf_ps = ps_bigK.tile([128, 1024], F32, name="kfps", tag="bigK")
                        for cc in range(2):
                            ch = 2 * g + cc
                            nc.tensor.matmul(
                                kf_ps[:, 512 * cc : 512 * cc + 512],
                                kt_t[:, 128 * ch : 128 * ch + 128],
                                wk_t[:],
                                start=True,
                                stop=True,
                            )
                        ek = etK_pool.tile([128, 1024], F16, name="ek", tag="ek")
                        nc.scalar.activation(ek[:], kf_ps[:], AFT.Exp)
                        akk = etK_pool.tile([128, 1024], F16, name="akk", tag="akk")
                        nc.vector.tensor_scalar(akk[:], ek[:], 1.0, None, ALU.min)
                        kf = kf_pool.tile([128, 1024], F16, name="kf")
                        if g in (2, 5):
                            # B-variant: ACT drains the psum relu; DVE add is
                            # all-SBUF (2 elem/lane/cycle) - balances ACT/DVE
                            rk = etK_pool.tile([128, 1024], F16, name="rk", tag="rk")
                            nc.scalar.activation(rk[:], kf_ps[:], AFT.Relu)
                            nc.vector.tensor_tensor(kf[:], akk[:], rk[:], ALU.add)
                        else:
                            nc.vector.scalar_tensor_tensor(
                                kf[:], kf_ps[:], 0.0, akk[:], ALU.max, ALU.add
                            )
                        for j in range(2):
                            for cc in range(2):
                                ch = 2 * g + cc
                                nc.tensor.matmul(
                                    kv_ps[:, 256 * j : 256 * j + 256],
                                    v_ts[j][:, 65 * ch : 65 * ch + 65],
                                    kf[:, 512 * cc + 256 * j : 512 * cc + 256 * j + 256],
                                    start=(ch == 0 and j == 0),
                                    stop=(ch == NCHUNK - 1),
                                )
                    return go

                return [blk(g) for g in range(8)]

            def emit_kvt_dl(hp):
                s_ = st[hp]
                kv_ps = s_["kv_ps"]
                kstk = {}
                for j in range(2):
                    kvc_t = kvs_pool.tile([65, 256], F16, name=f"kvc{j}", tag=f"kvc{j}")
                    nc.scalar.mul(kvc_t[:], kv_ps[:, 256 * j : 256 * j + 256], ETA)
                    tp_ps = ps_sm.tile([128, 130], F32, name="tpps", tag="sm")
                    for c in range(C):
                        p, odd = c // 2, c % 2
                        nc.tensor.matmul(
                            tp_ps[64 * odd : 64 * odd + 64, 65 * p : 65 * p + 65],
                            kvc_t[:, 64 * c : 64 * c + 64],
                            ident[:65, :65],
                            start=True,
                            stop=True,
                        )
                    kvs_t = kvs_pool.tile([128, 130], F16, name=f"kvs{j}", tag=f"kvs{j}")
                    nc.scalar.copy(kvs_t[:], tp_ps[:])
                    for p in range(2):
                        kstk[(j, p)] = kvs_t[:, 65 * p : 65 * p + 65]
                s_["kstk"] = kstk

                dlw = {}
                for j in range(2):
                    for p in range(2):
                        dl = dl_pool.tile([128, 128], F16, name=f"dl{j}{p}", tag=f"dl{j}{p}")
                        nc.vector.memset(dl[:], 0.0)
                        base = 4 * j + 2 * p
                        ks = kstk[(j, p)][:, 64:65]
                        nc.vector.tensor_scalar(
                            dl[:, base : base + 1], ks, wmask_t[:, 3 * p : 3 * p + 1],
                            None, ALU.mult,
                        )
                        nc.vector.tensor_scalar(
                            dl[:, base + 1 : base + 2], ks,
                            wmask_t[:, 3 * p + 1 : 3 * p + 2], None, ALU.mult,
                        )
                        if j == 0 and p == 0:
                            nc.vector.tensor_scalar(
                                dl[:, 8:128], ks.to_broadcast((128, 120)),
                                wmask_t[:, 2:3], None, ALU.mult,
                            )
                        dlw[(j, p)] = dl
                s_["dlw"] = dlw

            def b_blocks(hp):
                s_ = st[hp]
                s_["qf"] = {}

                def blk(j, p, half):
                    def go():
                        if (j, p) not in s_["qf"]:
                            s_["qf"][(j, p)] = qf_pool.tile(
                                [128, S], F16, name=f"qf{j}{p}", tag="qf"
                            )
                        qf_t = s_["qf"][(j, p)]
                        pq_ps = ps_bigQ.tile([128, 1024], F32, name="pqps", tag="bigQ")
                        for g in range(2):
                            o = 1024 * half + 512 * g
                            nc.tensor.matmul(
                                pq_ps[:, 512 * g : 512 * g + 512],
                                wq_t[p][:],
                                s_["qd"][j][:, o : o + 512],
                                start=True,
                                stop=True,
                            )
                        eq = etQ_pool.tile([128, 1024], F16, name="eq", tag="eq")
                        nc.scalar.activation(eq[:], pq_ps[:], AFT.Exp)
                        aq = etQ_pool.tile([128, 1024], F16, name="aq", tag="aq")
                        nc.vector.tensor_scalar(aq[:], eq[:], 1.0, None, ALU.min)
                        if 4 * j + 2 * p + half in (2, 5):
                            rq = etQ_pool.tile([128, 1024], F16, name="rq", tag="rq")
                            nc.scalar.activation(rq[:], pq_ps[:], AFT.Relu)
                            nc.vector.tensor_tensor(
                                qf_t[:, 1024 * half : 1024 * half + 1024],
                                aq[:], rq[:], ALU.add,
                            )
                        else:
                            nc.vector.scalar_tensor_tensor(
                                qf_t[:, 1024 * half : 1024 * half + 1024],
                                pq_ps[:], 0.0, aq[:], ALU.max, ALU.add,
                            )
                    return go

                return [blk(j, p, half) for j in range(2) for p in range(2)
                        for half in range(2)]

            def c_blocks(hp):
                s_ = st[hp]
                h0 = 2 * hp

                def blk(ch4):
                    def go():
                        sl = slice(512 * ch4, 512 * ch4 + 512)
                        qf2, dlw, kstk = s_["qf"], s_["dlw"], s_["kstk"]
                        den_ps = ps_sm.tile([128, 512], F32, name="denps", tag="sm")
                        for i, (j, p) in enumerate(((0, 0), (0, 1), (1, 0), (1, 1))):
                            nc.tensor.matmul(
                                den_ps[:],
                                dlw[(j, p)][:],
                                qf2[(j, p)][:, sl],
                                start=(i == 0),
                                stop=(i == 3),
                            )
                        lnt = lnt_pool.tile([128, 512], F32, name="lnt", tag="lnt")
                        nc.scalar.activation(lnt[:], den_ps[:], AFT.Ln)
                        z_t = z_pool.tile([128, 512], F16, name="zt", tag="z")
                        nc.scalar.activation(z_t[:], lnt[:], AFT.Exp, scale=-1.0)

                        ot_ps = ps_out.tile([128, 512], F32, name="otps", tag="ot")
                        for j in range(2):
                            for p in range(2):
                                zrep_ps = ps_sm.tile([128, 512], F32, name="zrepps", tag="sm")
                                nc.tensor.matmul(
                                    zrep_ps[:], sel_t[2 * j + p][:], z_t[:],
                                    start=True, stop=True,
                                )
                                zq_t = zq_pool.tile([128, 512], F16, name="zqt", tag="zq")
                                nc.vector.tensor_tensor(
                                    zq_t[:], qf2[(j, p)][:, sl], zrep_ps[:], ALU.mult
                                )
                                nc.tensor.matmul(
                                    ot_ps[64 * j : 64 * j + 64, :],
                                    kstk[(j, p)][:, 0:64],
                                    zq_t[:],
                                    start=(p == 0),
                                    stop=(p == 1),
                                )
                        ob5 = ob_pool.tile([128, 512], F32, name="ob5", tag="ob")
                        nc.scalar.copy(ob5[:], ot_ps[:])
                        for j in range(2):
                            nc.sync.dma_start(
                                ap["outT"][h0 + j][:, sl], ob5[64 * j : 64 * j + 64, :]
                            )
                    return go

                return [blk(ch4) for ch4 in range(4)]

            # ---- pipelined emission: interleave C(hp) with A(hp+1) so each
            # engine's in-order stream always has runnable work ----
            emit_dmas(0)
            for b in a_blocks(0):
                b()
            emit_kvt_dl(0)
            for b in b_blocks(0):
                b()
            for hp in range(NHP - 1):
                nxt = hp + 1
                emit_dmas(nxt)
                ai = iter(a_blocks(nxt))
                for c in c_blocks(hp):
                    for _ in range(2):
                        a = next(ai, None)
                        if a:
                            a()
                    c()
                for a in ai:
                    a()
                emit_kvt_dl(nxt)
                if nxt < NHP - 1:
                    for b in b_blocks(nxt):
                        b()
            # last h-pair: weave B with C. C chunks 0-1 read only the first
            # half of each qf tile (written by half=0 blocks), chunks 2-3
            # the second half - so B halves can interleave with C.
            last = NHP - 1
            bb = b_blocks(last)
            cb = c_blocks(last)
            h0 = [bb[i] for i in (0, 2, 4, 6)]
            h1 = [bb[i] for i in (1, 3, 5, 7)]
            for b in h0:
                b()
            cb[0]()
            h1[0](); h1[1]()
            cb[1]()
            h1[2](); h1[3]()
            cb[2]()
            cb[3]()

    _split_multiwait(nc)
    return nc


_NC_CACHE = None


def _get_nc():
    global _NC_CACHE
    if _NC_CACHE is None:
        _NC_CACHE = build_program()
    return _NC_CACHE


def _softmax(x):
    e = np.exp(x - x.max())
    return e / e.sum()


def prep_core_inputs(queries, keys, values, key_mask, feat_W, mix_weights, core):
    n, hh = core // 2, (core % 2) * HL
    W = _softmax(np.asarray(mix_weights, np.float64)).astype(np.float32)

    qs = queries[n][:, hh : hh + HL, :].transpose(1, 2, 0)  # [HL, D, S]
    qTd = np.ascontiguousarray(
        np.concatenate([qs, qs], axis=1)
    ).astype(np.float16)  # [HL, 128, S]
    ks = keys[n][:, hh : hh + HL, :].transpose(1, 2, 0)
    kT = np.ascontiguousarray(ks.reshape(NHP, 128, S)).astype(np.float16)

    mask = key_mask[n].astype(np.float32)
    vm = values[n][:, hh : hh + HL, :] * mask[:, None, None]
    vaug = np.concatenate(
        [vm, np.broadcast_to(mask[:, None, None], (S, HL, 1))], axis=2
    )
    vaug = vaug.transpose(1, 0, 2).reshape(HL, NCHUNK, 128, 65)
    vaug = np.ascontiguousarray(vaug.transpose(0, 2, 1, 3)).reshape(
        HL, 128, NCHUNK * 65
    ).astype(np.float16)

    wq2 = np.zeros((2, 128, 128), np.float16)
    for p in range(2):
        wq2[p, :64, :64] = feat_W[2 * p]
        wq2[p, 64:, 64:] = feat_W[2 * p + 1]
    wcat = np.concatenate([feat_W[c] for c in range(C)], axis=1)
    wk = np.zeros((128, 512), np.float16)
    wk[:64, :256] = wcat
    wk[64:, 256:] = wcat

    wmask = np.zeros((128, 6), np.float32)
    for p in range(2):
        wmask[:64, 3 * p + 0] = 1.0 / W[2 * p]
        wmask[64:, 3 * p + 1] = 1.0 / W[2 * p + 1]
        wmask[:64, 3 * p + 2] = 1.0 / W[2 * p]
        wmask[64:, 3 * p + 2] = 1.0 / W[2 * p + 1]

    sel2 = np.zeros((4, 128, 128), np.float16)
    for j in range(2):
        for p in range(2):
            base = 4 * j + 2 * p
            sel2[2 * j + p, base, :64] = 1.0
            sel2[2 * j + p, base + 1, 64:] = 1.0

    return {"qTd": qTd, "kT": kT, "vaug": vaug, "wq2": wq2, "wk": wk,
            "wmask": wmask, "sel2": sel2}


def run_cores(inputs, trace=False, tmpdir=None):
    from concourse.bass_utils import run_bass_kernel_spmd

    nc = _get_nc()
    in_maps = [prep_core_inputs(**inputs, core=i) for i in range(8)]
    kwargs = {}
    if trace:
        kwargs = {"trace": True, "tmpdir": tmpdir}
    res = run_bass_kernel_spmd(nc, in_maps, core_ids=list(range(8)), **kwargs)
    out = np.empty((N, S, H, M), np.float32)
    for i in range(8):
        n, hh = i // 2, (i % 2) * HL
        oT = res.results[i]["outT"]  # [HL, 64, S]
        for h in range(HL):
            out[n, :, hh + h, :] = oT[h].T
    return out, res


def kernel(queries, keys, values, key_mask, feat_W, mix_weights):
    out, _ = run_cores(
        dict(queries=np.asarray(queries), keys=np.asarray(keys),
             values=np.asarray(values), key_mask=np.asarray(key_mask),
             feat_W=np.asarray(feat_W), mix_weights=np.asarray(mix_weights))
    )
    return out


# revision 47
# speedup vs baseline: 1.0748x; 1.0045x over previous
"""MixtureLinearAttention TRN2 kernel (8 NeuronCores, SPMD).

Math (per batch n, component c, head h):
  Qf = elu(q @ W_c) + 1 ;  Kf = (elu(k @ W_c) + 1) * mask
  KVt[e, m] = sum_s Kf[s,e] V[s,m] ;  Ksum[e] = sum_s Kf[s,e]
  Den[s] = sum_e Qf[s,e] Ksum[e]
  out[s,h,m] = sum_c softmax(mix)_c / Den[s] * sum_e Qf[s,e] KVt[e,m]

Sharding: core i -> (n = i//2, heads hh = (i%2)*8..+8). Host does all layout
transposes (q/k fed d-major; output returned m-major and transposed back).

v6 notes vs baseline:
- Matmul operands fp16 (halves ldweights cost; PSUM stays fp32). KV stack
  scaled by ETA=2^-12 at the psum->sbuf copy so z=1/den and zq stay in fp16
  normal range; ETA cancels exactly in the out matmul.
- PSUM-draining copies ride the scalar engine (ACT reads PSUM at full rate);
  DVE keeps the elu finish (fp16 min + scalar_tensor_tensor) and zq.
- KVt transpose matmuls write c-pair stacks at partition offset 64 directly;
  out psum packed [128,512] (head j at partition 64*j).
- Software-pipelined emission: phase C (den/z/zrep/zq/out) of h-pair hp is
  interleaved with phase A (phi-K/KV) of hp+1 so each engine's in-order
  stream always has runnable work (engines execute their streams in order).
"""
import sys

if "/opt/trn_rl_repo" not in sys.path:
    sys.path.insert(0, "/opt/trn_rl_repo")

from contextlib import ExitStack

import numpy as np

import concourse.bass as bass
import concourse.tile as tile
from concourse import mybir
from concourse.masks import make_identity

F32 = mybir.dt.float32
F16 = mybir.dt.float16
ALU = mybir.AluOpType
AFT = mybir.ActivationFunctionType

N, S, H, D, C = 4, 2048, 16, 64, 4
E = M = 64
HL = 8          # heads per core
NHP = HL // 2   # h-pairs
NCHUNK = S // 128
ETA = 2.0 ** -12


def _split_multiwait(nc, max_waits=1):
    """This walrus build rejects >1 sync wait per instruction; hoist extra
    waits onto NoOps inserted just before, on the same engine."""
    k = 0
    for fn in nc.m.functions:
        for bb in fn.blocks:
            out, changed = [], False
            for inst in bb.instructions:
                si = inst.sync_info
                if si is not None and si.on_wait and len(si.on_wait) > max_waits:
                    waits = list(si.on_wait)
                    while len(waits) > max_waits:
                        chunk, waits = waits[:max_waits], waits[max_waits:]
                        nop = mybir.InstNoOp(name=f"wait_split_{k}", ins=[], outs=[])
                        k += 1
                        nop.engine = inst.engine
                        nop.sync_info = mybir.SyncInfo(on_wait=chunk, on_update=[])
                        out.append(nop)
                        changed = True
                    inst.sync_info = mybir.SyncInfo(
                        on_wait=waits, on_update=list(si.on_update or [])
                    )
                out.append(inst)
            if changed:
                bb.instructions = out


def build_program():
    nc = bass.Bass("TRN2", debug=False)
    ap = {}
    ap["qTd"] = nc.dram_tensor("qTd", [HL, 128, S], F16, kind="ExternalInput").ap()
    ap["kT"] = nc.dram_tensor("kT", [NHP, 128, S], F16, kind="ExternalInput").ap()
    ap["vaug"] = nc.dram_tensor("vaug", [HL, 128, NCHUNK * 65], F16, kind="ExternalInput").ap()
    ap["wq2"] = nc.dram_tensor("wq2", [2, 128, 128], F16, kind="ExternalInput").ap()
    ap["wk"] = nc.dram_tensor("wk", [128, 512], F16, kind="ExternalInput").ap()
    ap["wmask"] = nc.dram_tensor("wmask", [128, 6], F32, kind="ExternalInput").ap()
    ap["sel2"] = nc.dram_tensor("sel2", [4, 128, 128], F16, kind="ExternalInput").ap()
    ap["outT"] = nc.dram_tensor("outT", [HL, 64, S], F32, kind="ExternalOutput").ap()

    tc = tile.TileContext(nc)
    with tc:
        with ExitStack() as ctx:
            cpool = ctx.enter_context(tc.tile_pool(name="consts", bufs=1))
            wq_t = []
            for p in range(2):
                w1 = cpool.tile([128, 128], F16, name=f"wq{p}", tag=f"wq{p}")
                nc.sync.dma_start(w1[:], ap["wq2"][p])
                wq_t.append(w1)
            wk_t = cpool.tile([128, 512], F16)
            nc.sync.dma_start(wk_t[:], ap["wk"][:])
            wmask_t = cpool.tile([128, 6], F32)
            nc.sync.dma_start(wmask_t[:], ap["wmask"][:])
            sel_t = []
            for i in range(4):
                s1 = cpool.tile([128, 128], F16, name=f"sel{i}", tag=f"sel{i}")
                nc.sync.dma_start(s1[:], ap["sel2"][i])
                sel_t.append(s1)
            ident = cpool.tile([128, 128], F16)
            make_identity(nc, ident[:])

            qd_pool = ctx.enter_context(tc.tile_pool(name="qd", bufs=1))
            kt_pool = ctx.enter_context(tc.tile_pool(name="kt", bufs=2))
            v_pool = ctx.enter_context(tc.tile_pool(name="v", bufs=2))
            kf_pool = ctx.enter_context(tc.tile_pool(name="kf", bufs=8))
            etK_pool = ctx.enter_context(tc.tile_pool(name="etK", bufs=2))
            etQ_pool = ctx.enter_context(tc.tile_pool(name="etQ", bufs=2))
            qf_pool = ctx.enter_context(tc.tile_pool(name="qf", bufs=5))
            kvs_pool = ctx.enter_context(tc.tile_pool(name="kvs", bufs=2))
            dl_pool = ctx.enter_context(tc.tile_pool(name="dl", bufs=2))
            z_pool = ctx.enter_context(tc.tile_pool(name="z", bufs=2))
            lnt_pool = ctx.enter_context(tc.tile_pool(name="lnt", bufs=2))
            zq_pool = ctx.enter_context(tc.tile_pool(name="zq", bufs=2))
            ob_pool = ctx.enter_context(tc.tile_pool(name="ob", bufs=3))
            # PSUM (8 banks): bigK 2 + bigQ 2 + kv 1 + sm 2 + out 1
            ps_bigK = ctx.enter_context(tc.tile_pool(name="psbigK", bufs=1, space="PSUM"))
            ps_bigQ = ctx.enter_context(tc.tile_pool(name="psbigQ", bufs=1, space="PSUM"))
            ps_kv = ctx.enter_context(tc.tile_pool(name="pskv", bufs=1, space="PSUM"))
            ps_sm = ctx.enter_context(tc.tile_pool(name="pssm", bufs=2, space="PSUM"))
            ps_out = ctx.enter_context(tc.tile_pool(name="psout", bufs=1, space="PSUM"))

            st = [dict() for _ in range(NHP)]

            def emit_dmas(hp):
                h0 = 2 * hp
                kt_t = kt_pool.tile([128, S], F16, name="kt")
                nc.sync.dma_start(kt_t[:], ap["kT"][hp])
                v_ts, qd_ts = [], []
                for j in range(2):
                    v_t = v_pool.tile([128, NCHUNK * 65], F16, name=f"v{j}", tag=f"v{j}")
                    nc.sync.dma_start(v_t[:], ap["vaug"][h0 + j])
                    v_ts.append(v_t)
                for j in range(2):
                    qd_t = qd_pool.tile([128, S], F16, name=f"qd{j}", tag=f"qd{j}")
                    nc.sync.dma_start(qd_t[:], ap["qTd"][h0 + j])
                    qd_ts.append(qd_t)
                st[hp].update(kt=kt_t, v=v_ts, qd=qd_ts)

            def a_blocks(hp):
                s_ = st[hp]
                s_["kv_ps"] = None
                s_["kf"] = {}

                def blk(g):
                    def go():
                        if s_["kv_ps"] is None:
                            s_["kv_ps"] = ps_kv.tile([65, 512], F32, name="kvps", tag="kvps")
                        kv_ps = s_["kv_ps"]
                        kt_t, v_ts = s_["kt"], s_["v"]
                        kf_ps = ps_bigK.tile([128, 1024], F32, name="kfps", tag="bigK")
                        for cc in range(2):
                            ch = 2 * g + cc
                            nc.tensor.matmul(
                                kf_ps[:, 512 * cc : 512 * cc + 512],
                                kt_t[:, 128 * ch : 128 * ch + 128],
                                wk_t[:],
                                start=True,
                                stop=True,
                            )
                        ek = etK_pool.tile([128, 1024], F16, name="ek", tag="ek")
                        nc.scalar.activation(ek[:], kf_ps[:], AFT.Exp)
                        akk = etK_pool.tile([128, 1024], F16, name="akk", tag="akk")
                        nc.vector.tensor_scalar(akk[:], ek[:], 1.0, None, ALU.min)
                        kf = kf_pool.tile([128, 1024], F16, name="kf")
                        if g in (2, 5):
                            # B-variant: ACT drains the psum relu; DVE add is
                            # all-SBUF (2 elem/lane/cycle) - balances ACT/DVE
                            rk = etK_pool.tile([128, 1024], F16, name="rk", tag="rk")
                            nc.scalar.activation(rk[:], kf_ps[:], AFT.Relu)
                            nc.vector.tensor_tensor(kf[:], akk[:], rk[:], ALU.add)
                        else:
                            nc.vector.scalar_tensor_tensor(
                                kf[:], kf_ps[:], 0.0, akk[:], ALU.max, ALU.add
                            )
                        for j in range(2):
                            for cc in range(2):
                                ch = 2 * g + cc
                                nc.tensor.matmul(
                                    kv_ps[:, 256 * j : 256 * j + 256],
                                    v_ts[j][:, 65 * ch : 65 * ch + 65],
                                    kf[:, 512 * cc + 256 * j : 512 * cc + 256 * j + 256],
                                    start=(ch == 0 and j == 0),
                                    stop=(ch == NCHUNK - 1),
                                )
                    return go

                return [blk(g) for g in range(8)]

            def emit_kvt_dl(hp):
                s_ = st[hp]
                kv_ps = s_["kv_ps"]
                kstk = {}
                for j in range(2):
                    kvc_t = kvs_pool.tile([65, 256], F16, name=f"kvc{j}", tag=f"kvc{j}")
                    nc.scalar.mul(kvc_t[:], kv_ps[:, 256 * j : 256 * j + 256], ETA)
                    tp_ps = ps_sm.tile([128, 130], F32, name="tpps", tag="sm")
                    for c in range(C):
                        p, odd = c // 2, c % 2
                        nc.tensor.matmul(
                            tp_ps[64 * odd : 64 * odd + 64, 65 * p : 65 * p + 65],
                            kvc_t[:, 64 * c : 64 * c + 64],
                            ident[:65, :65],
                            start=True,
                            stop=True,
                        )
                    kvs_t = kvs_pool.tile([128, 130], F16, name=f"kvs{j}", tag=f"kvs{j}")
                    nc.scalar.copy(kvs_t[:], tp_ps[:])
                    for p in range(2):
                        kstk[(j, p)] = kvs_t[:, 65 * p : 65 * p + 65]
                s_["kstk"] = kstk

                dlw = {}
                for j in range(2):
                    for p in range(2):
                        dl = dl_pool.tile([128, 128], F16, name=f"dl{j}{p}", tag=f"dl{j}{p}")
                        nc.vector.memset(dl[:], 0.0)
                        base = 4 * j + 2 * p
                        ks = kstk[(j, p)][:, 64:65]
                        nc.vector.tensor_scalar(
                            dl[:, base : base + 1], ks, wmask_t[:, 3 * p : 3 * p + 1],
                            None, ALU.mult,
                        )
                        nc.vector.tensor_scalar(
                            dl[:, base + 1 : base + 2], ks,
                            wmask_t[:, 3 * p + 1 : 3 * p + 2], None, ALU.mult,
                        )
                        if j == 0 and p == 0:
                            nc.vector.tensor_scalar(
                                dl[:, 8:128], ks.to_broadcast((128, 120)),
                                wmask_t[:, 2:3], None, ALU.mult,
                            )
                        dlw[(j, p)] = dl
                s_["dlw"] = dlw

            def b_blocks(hp):
                s_ = st[hp]
                s_["qf"] = {}

                def blk(j, p, half):
                    def go():
                        if (j, p) not in s_["qf"]:
                            s_["qf"][(j, p)] = qf_pool.tile(
                                [128, S], F16, name=f"qf{j}{p}", tag="qf"
                            )
                        qf_t = s_["qf"][(j, p)]
                        pq_ps = ps_bigQ.tile([128, 1024], F32, name="pqps", tag="bigQ")
                        for g in range(2):
                            o = 1024 * half + 512 * g
                            nc.tensor.matmul(
                                pq_ps[:, 512 * g : 512 * g + 512],
                                wq_t[p][:],
                                s_["qd"][j][:, o : o + 512],
                                start=True,
                                stop=True,
                            )
                        eq = etQ_pool.tile([128, 1024], F16, name="eq", tag="eq")
                        nc.scalar.activation(eq[:], pq_ps[:], AFT.Exp)
                        aq = etQ_pool.tile([128, 1024], F16, name="aq", tag="aq")
                        nc.vector.tensor_scalar(aq[:], eq[:], 1.0, None, ALU.min)
                        if 4 * j + 2 * p + half in (2, 5):
                            rq = etQ_pool.tile([128, 1024], F16, name="rq", tag="rq")
                            nc.scalar.activation(rq[:], pq_ps[:], AFT.Relu)
                            nc.vector.tensor_tensor(
                                qf_t[:, 1024 * half : 1024 * half + 1024],
                                aq[:], rq[:], ALU.add,
                            )
                        else:
                            nc.vector.scalar_tensor_tensor(
                                qf_t[:, 1024 * half : 1024 * half + 1024],
                                pq_ps[:], 0.0, aq[:], ALU.max, ALU.add,
                            )
                    return go

                return [blk(j, p, half) for j in range(2) for p in range(2)
                        for half in range(2)]

            def c_blocks(hp):
                s_ = st[hp]
                h0 = 2 * hp

                def blk(ch4):
                    def go():
                        sl = slice(512 * ch4, 512 * ch4 + 512)
                        qf2, dlw, kstk = s_["qf"], s_["dlw"], s_["kstk"]
                        den_ps = ps_sm.tile([128, 512], F32, name="denps", tag="sm")
                        for i, (j, p) in enumerate(((0, 0), (0, 1), (1, 0), (1, 1))):
                            nc.tensor.matmul(
                                den_ps[:],
                                dlw[(j, p)][:],
                                qf2[(j, p)][:, sl],
                                start=(i == 0),
                                stop=(i == 3),
                            )
                        lnt = lnt_pool.tile([128, 512], F32, name="lnt", tag="lnt")
                        nc.scalar.activation(lnt[:], den_ps[:], AFT.Ln)
                        z_t = z_pool.tile([128, 512], F16, name="zt", tag="z")
                        nc.scalar.activation(z_t[:], lnt[:], AFT.Exp, scale=-1.0)

                        ot_ps = ps_out.tile([128, 512], F32, name="otps", tag="ot")
                        for j in range(2):
                            for p in range(2):
                                zrep_ps = ps_sm.tile([128, 512], F32, name="zrepps", tag="sm")
                                nc.tensor.matmul(
                                    zrep_ps[:], sel_t[2 * j + p][:], z_t[:],
                                    start=True, stop=True,
                                )
                                zq_t = zq_pool.tile([128, 512], F16, name="zqt", tag="zq")
                                nc.vector.tensor_tensor(
                                    zq_t[:], qf2[(j, p)][:, sl], zrep_ps[:], ALU.mult
                                )
                                nc.tensor.matmul(
                                    ot_ps[64 * j : 64 * j + 64, :],
                                    kstk[(j, p)][:, 0:64],
                                    zq_t[:],
                                    start=(p == 0),
                                    stop=(p == 1),
                                )
                        ob5 = ob_pool.tile([128, 512], F32, name="ob5", tag="ob")
                        nc.scalar.copy(ob5[:], ot_ps[:])
                        for j in range(2):
                            nc.sync.dma_start(
                                ap["outT"][h0 + j][:, sl], ob5[64 * j : 64 * j + 64, :]
                            )
                    return go

                return [blk(ch4) for ch4 in range(4)]

            # ---- pipelined emission: interleave C(hp) with A(hp+1) so each
            # engine's in-order stream always has runnable work ----
            emit_dmas(0)
            for b in a_blocks(0):
                b()
            emit_kvt_dl(0)
            # Weave each h-pair's B with its own C (C chunks 0-1 read only
            # the first half of each qf tile, chunks 2-3 the second half)
            # and with the next h-pair's A blocks - one continuous pipeline.
            for hp in range(NHP):
                nxt = hp + 1
                ab = []
                if nxt < NHP:
                    emit_dmas(nxt)
                    ab = a_blocks(nxt)
                ai = iter(ab)

                def a1():
                    a = next(ai, None)
                    if a:
                        a()

                bb = b_blocks(hp)
                cb = c_blocks(hp)
                h0 = [bb[i] for i in (0, 2, 4, 6)]
                h1 = [bb[i] for i in (1, 3, 5, 7)]
                for b in h0:
                    b()
                    a1()
                cb[0]()
                h1[0](); a1(); h1[1](); a1()
                cb[1]()
                h1[2](); a1(); h1[3](); a1()
                cb[2]()
                for a in ai:
                    a()
                cb[3]()
                if nxt < NHP:
                    emit_kvt_dl(nxt)

    _split_multiwait(nc)
    return nc


_NC_CACHE = None


def _get_nc():
    global _NC_CACHE
    if _NC_CACHE is None:
        _NC_CACHE = build_program()
    return _NC_CACHE


def _softmax(x):
    e = np.exp(x - x.max())
    return e / e.sum()


def prep_core_inputs(queries, keys, values, key_mask, feat_W, mix_weights, core):
    n, hh = core // 2, (core % 2) * HL
    W = _softmax(np.asarray(mix_weights, np.float64)).astype(np.float32)

    qs = queries[n][:, hh : hh + HL, :].transpose(1, 2, 0)  # [HL, D, S]
    qTd = np.ascontiguousarray(
        np.concatenate([qs, qs], axis=1)
    ).astype(np.float16)  # [HL, 128, S]
    ks = keys[n][:, hh : hh + HL, :].transpose(1, 2, 0)
    kT = np.ascontiguousarray(ks.reshape(NHP, 128, S)).astype(np.float16)

    mask = key_mask[n].astype(np.float32)
    vm = values[n][:, hh : hh + HL, :] * mask[:, None, None]
    vaug = np.concatenate(
        [vm, np.broadcast_to(mask[:, None, None], (S, HL, 1))], axis=2
    )
    vaug = vaug.transpose(1, 0, 2).reshape(HL, NCHUNK, 128, 65)
    vaug = np.ascontiguousarray(vaug.transpose(0, 2, 1, 3)).reshape(
        HL, 128, NCHUNK * 65
    ).astype(np.float16)

    wq2 = np.zeros((2, 128, 128), np.float16)
    for p in range(2):
        wq2[p, :64, :64] = feat_W[2 * p]
        wq2[p, 64:, 64:] = feat_W[2 * p + 1]
    wcat = np.concatenate([feat_W[c] for c in range(C)], axis=1)
    wk = np.zeros((128, 512), np.float16)
    wk[:64, :256] = wcat
    wk[64:, 256:] = wcat

    wmask = np.zeros((128, 6), np.float32)
    for p in range(2):
        wmask[:64, 3 * p + 0] = 1.0 / W[2 * p]
        wmask[64:, 3 * p + 1] = 1.0 / W[2 * p + 1]
        wmask[:64, 3 * p + 2] = 1.0 / W[2 * p]
        wmask[64:, 3 * p + 2] = 1.0 / W[2 * p + 1]

    sel2 = np.zeros((4, 128, 128), np.float16)
    for j in range(2):
        for p in range(2):
            base = 4 * j + 2 * p
            sel2[2 * j + p, base, :64] = 1.0
            sel2[2 * j + p, base + 1, 64:] = 1.0

    return {"qTd": qTd, "kT": kT, "vaug": vaug, "wq2": wq2, "wk": wk,
            "wmask": wmask, "sel2": sel2}


def run_cores(inputs, trace=False, tmpdir=None):
    from concourse.bass_utils import run_bass_kernel_spmd

    nc = _get_nc()
    in_maps = [prep_core_inputs(**inputs, core=i) for i in range(8)]
    kwargs = {}
    if trace:
        kwargs = {"trace": True, "tmpdir": tmpdir}
    res = run_bass_kernel_spmd(nc, in_maps, core_ids=list(range(8)), **kwargs)
    out = np.empty((N, S, H, M), np.float32)
    for i in range(8):
        n, hh = i // 2, (i % 2) * HL
        oT = res.results[i]["outT"]  # [HL, 64, S]
        for h in range(HL):
            out[n, :, hh + h, :] = oT[h].T
    return out, res


def kernel(queries, keys, values, key_mask, feat_W, mix_weights):
    out, _ = run_cores(
        dict(queries=np.asarray(queries), keys=np.asarray(keys),
             values=np.asarray(values), key_mask=np.asarray(key_mask),
             feat_W=np.asarray(feat_W), mix_weights=np.asarray(mix_weights))
    )
    return out
